# revision 26
# baseline (speedup 1.0000x reference)
"""Trainium2 Bass kernel for nn_NewActivationGNN (GNN message passing).

Architecture (v2, swdge gather):
  y_l = A_norm @ (x @ W_l) is computed as (A_norm @ x) @ W_l — aggregate
  raw hidden rows first, transform after. Per-edge source rows are pulled
  straight from the DRAM-resident AllGathered node table with SWDGE
  dma_gather (one 256B descriptor per edge, slot-major destination), so
  the gather costs ~23ns/edge across 16 SDMA engines instead of ~26ns/edge
  serialized on gpsimd Q7 cores.

  Slot-major pipeline: gathered chunks [128 edge-slots, 128 hid] are lhsT
  for selection-matrix matmuls (rhs = S fp16 [slot, dest-span] carrying
  deg_inv) accumulating feature-major y_raw per 512-dest pack in PSUM;
  the W_l transform (lhsT = y_raw chunk, rhs = W) lands slot-major
  [128 dest, 128 hid] tiles; activation + residual run slot-major
  full-width; the next table is DMA'd out row-major and AllGathered.

  S matrices and the index stream are small (dest-dense cells of
  (pack, src-half)) and stay SBUF-resident across all 4 layers.

Sharding: nodes split across 8 cores by destination (graph parallel);
per-layer AllGather of the fp16 slot-major table; weights replicated.
SPMD: one program for all cores; per-device variation (indices, S
matrices, features) is data. Chunk counts / S spans are made
device-uniform by padding to the cross-device maximum.

Edge indices are int16, so the node table is addressed in two halves
(src position < 32768 vs >= 32768); cells are keyed (pack, half).
"""

import sys

for _p in ("/opt/trn_rl_repo", "/root/.axon_site/_ro/trn_rl_repo"):
    if _p not in sys.path:
        sys.path.insert(0, _p)

from dataclasses import dataclass

import numpy as np

import concourse.bass as bass  # noqa: F401
import concourse.tile as tile
from concourse import bacc, mybir
from concourse.masks import make_identity

P = 128
SPLITA = 3072             # rows per device in table half A (24 tiles)



@dataclass
class Cfg:
    N: int = 50000
    E: int = 800000
    NFEAT: int = 500
    NHID: int = 128
    NCLASS: int = 40
    NLAYERS: int = 4
    GAMMA: float = 0.3
    X1: float = 0.1
    X2: float = 0.9
    C_ACT: float = -1.0
    n_cores: int = 8
    PACK: int = 512

    @property
    def R(self):
        return self.N // self.n_cores          # 6250 dest rows per core

    @property
    def NPACK(self):
        return (self.R + self.PACK - 1) // self.PACK   # 13

    @property
    def NFP(self):
        return ((self.NFEAT + 1 + P - 1) // P) * P     # 512


class Sched:
    """Device-uniform schedule over cells keyed (pack k, half h)."""

    def __init__(self, cfg, counts, spans):
        self.B = counts.max(axis=0)                      # [NPACK, 2]
        self.nch = (self.B + P - 1) // P
        # idx-stream cell order: delay each half-B cell by DELAY packs so
        # half-A gathers run while the B AllGather is still in flight
        DELAY = 4
        self.cell_order = []
        for k in range(cfg.NPACK):
            self.cell_order.append((k, 0))
            if k >= DELAY:
                self.cell_order.append((k - DELAY, 1))
        for k in range(cfg.NPACK - DELAY, cfg.NPACK):
            self.cell_order.append((k, 1))
        self.cell_off = {}
        off = 0
        for (k, h) in self.cell_order:
            self.cell_off[(k, h)] = off
            off += int(self.nch[k, h]) * P
        self.idx_total = off
        s_off = 0
        self.s_cell = {}
        for k in range(cfg.NPACK):
            for h in range(2):
                metas = []
                for ci in range(int(self.nch[k, h])):
                    co, w = spans[(k, h, ci)]
                    metas.append((co, w, s_off))
                    s_off += w
                self.s_cell[(k, h)] = metas
        self.s_total = s_off


def preprocess(cfg: Cfg, features, edge_row, edge_col, W_in, Ws, c, W_out):
    N, R, ncores = cfg.N, cfg.R, cfg.n_cores
    NPACK, PACK = cfg.NPACK, cfg.PACK
    f32 = np.float32

    deg = np.bincount(edge_row, minlength=N)
    deg_inv = (1.0 / np.maximum(deg, 1)).astype(f32)
    owner = edge_row // R

    # Node permutation: within each device, order nodes by descending global
    # degree so cumulative degree profiles align across devices (smaller S
    # spans, less cell padding). pos[] maps orig node id -> its table
    # position within the owning device's slice.
    dest_of = np.empty((ncores, R), np.int64)   # sorted pos -> orig local
    pos = np.empty(N, np.int64)
    for d in range(ncores):
        order = np.argsort(-deg[d * R:(d + 1) * R], kind="stable")
        dest_of[d] = order
        local_of = np.empty(R, np.int64)
        local_of[order] = np.arange(R)
        pos[d * R:(d + 1) * R] = local_of

    dev = []
    counts = np.zeros((ncores, NPACK, 2), np.int64)
    for d in range(ncores):
        m = owner == d
        dl = pos[edge_row[m]]
        src = edge_col[m].astype(np.int64)
        sd = src // R
        r = pos[src]                            # row within source device
        h = (r >= SPLITA).astype(np.int64)      # table half (A/B)
        splitb = R - SPLITA
        iv = np.where(h == 0, sd * SPLITA + r,
                      sd * splitb + (r - SPLITA))  # int16-safe row index
        k = dl // PACK
        so = np.lexsort((dl, h, k))
        dl, h, iv, k = dl[so], h[so], iv[so], k[so]
        cell_id = k * 2 + h
        cnt = np.bincount(cell_id, minlength=NPACK * 2)
        counts[d] = cnt.reshape(NPACK, 2)
        dev.append((dl, iv, cell_id))

    B = counts.max(axis=0)
    nch = (B + P - 1) // P

    spans = {}
    dev_cell_start = []
    for d in range(ncores):
        cnt = counts[d].reshape(-1)
        dev_cell_start.append(np.concatenate([[0], np.cumsum(cnt)]))
    for k in range(NPACK):
        for h in range(2):
            cid = k * 2 + h
            for ci in range(int(nch[k, h])):
                fd, ld = [], []
                for d in range(ncores):
                    dl = dev[d][0]
                    b = dev_cell_start[d][cid]
                    n_d = counts[d, k, h]
                    s0 = ci * P
                    if s0 < n_d:
                        s1 = min(s0 + P, n_d)
                        fd.append(int(dl[b + s0]))
                        ld.append(int(dl[b + s1 - 1]))
                co = min(fd) - k * PACK
                w = max(ld) - min(fd) + 1
                spans[(k, h, ci)] = (co, w)

    sch = Sched(cfg, counts, spans)

    # shared weights
    NFP = cfg.NFP
    W_aug = np.zeros((NFP, cfg.NHID), f32)
    W_aug[:cfg.NFEAT] = (1.0 - cfg.GAMMA) * W_in
    W_aug[cfg.NFEAT] = cfg.GAMMA * np.maximum(c, 0.0)
    nk = NFP // P
    W_dram = np.empty((P, nk * P), np.float16)
    for kk in range(nk):
        W_dram[:, kk * P:(kk + 1) * P] = W_aug[kk * P:(kk + 1) * P]
    Ws_dram = np.empty((P, cfg.NLAYERS * P), np.float16)
    for l in range(cfg.NLAYERS):
        Ws_dram[:, l * P:(l + 1) * P] = Ws[l]
    Wout_dram = np.ascontiguousarray(W_out).astype(np.float16)

    in_maps = []
    for d in range(ncores):
        dl, iv, cell_id = dev[d]
        cstart = dev_cell_start[d]
        idx_vals = np.zeros(sch.idx_total, np.int16)
        s_data = np.zeros((P, sch.s_total), np.float16)
        for k in range(NPACK):
            for h in range(2):
                cid = k * 2 + h
                n_d = int(counts[d, k, h])
                if n_d == 0:
                    continue
                b = cstart[cid]
                o = sch.cell_off[(k, h)]
                idx_vals[o:o + n_d] = iv[b:b + n_d].astype(np.int16)
                metas = sch.s_cell[(k, h)]
                sl = np.arange(n_d)
                ci_arr = sl // P
                row = sl % P
                co = np.array([m[0] for m in metas], np.int64)[ci_arr]
                soff = np.array([m[2] for m in metas], np.int64)[ci_arr]
                col = soff + (dl[b:b + n_d] - k * PACK - co)
                s_data[row, col] = deg_inv[
                    d * R + dest_of[d][dl[b:b + n_d]]]
        idx_t = np.tile(idx_vals.reshape(-1, 16).T, (8, 1))

        gids = d * R + dest_of[d]
        featT = np.zeros((NFP, R), np.float16)
        featT[:cfg.NFEAT] = features[gids].T
        featT[cfg.NFEAT] = 1.0

        in_maps.append(dict(
            featT=featT, idx_all=np.ascontiguousarray(idx_t), s_all=s_data,
            w_proj=W_dram, w_hid=Ws_dram, w_out=Wout_dram,
        ))
    return in_maps, sch, dest_of


def build_program(cfg: Cfg, sch: Sched):
    nc = bacc.Bacc("TRN2", target_bir_lowering=False, debug=False,
                   num_devices=cfg.n_cores, num_swdge_queues=4)
    DT = mybir.dt.float16
    f32 = mybir.dt.float32
    R, NPACK, NFP = cfg.R, cfg.NPACK, cfg.NFP
    AFT = mybir.ActivationFunctionType
    ALU = mybir.AluOpType
    AX = mybir.AxisListType
    RED = bass.bass_isa.ReduceOp
    rg = [list(range(cfg.n_cores))]
    nk = NFP // P
    NCHK = (R + cfg.PACK - 1) // cfg.PACK   # 512-col chunks over R
    NT = R // P + (1 if R % P else 0)       # 128-col tiles over R (49)

    def cw(ch):
        return min(cfg.PACK, R - ch * cfg.PACK)

    featT = nc.dram_tensor("featT", [NFP, R], DT, kind="ExternalInput").ap()
    idx_all = nc.dram_tensor("idx_all", [P, sch.idx_total // 16],
                             mybir.dt.int16, kind="ExternalInput").ap()
    s_all = nc.dram_tensor("s_all", [P, sch.s_total], DT,
                           kind="ExternalInput").ap()
    w_proj = nc.dram_tensor("w_proj", [P, nk * P], DT,
                            kind="ExternalInput").ap()
    w_hid = nc.dram_tensor("w_hid", [P, cfg.NLAYERS * P], DT,
                           kind="ExternalInput").ap()
    w_out = nc.dram_tensor("w_out", [P, cfg.NCLASS], DT,
                           kind="ExternalInput").ap()
    out = nc.dram_tensor("out", [R, cfg.NCLASS], f32,
                         kind="ExternalOutput").ap()

    INV08 = float(np.float32(1.0 / (np.float64(cfg.X2) - cfg.X1 + 1e-8)))
    B_RELU = float(np.float32(-cfg.X1 * INV08))
    E1 = float(1.0 + np.exp(-cfg.C_ACT))
    NIT = cfg.NLAYERS
    FT = R // P          # full 128-row tiles (48)
    TAIL = R - FT * P    # ragged tail rows (106)
    SMW = NT * P         # slot-major width (6272)
    SPLITB = R - SPLITA  # rows per device in table half B (3178)

    # gather calls: group cells into calls of <= GCAP slots (contiguous in
    # the idx stream). Each call is one dma_gather against one table half.
    # GCAP is bounded by the SWDGE descriptor-ring carveout: one call's
    # descriptors (num_idxs/16 + 1 per DMA engine) must fit the ring or the
    # Q7 waits forever (hardware-verified: 1024 ok, 2560 wedges).
    GCAP = 1024
    calls = []     # (half, slot_off, n_slots)
    for (k, h) in sch.cell_order:
        n = int(sch.nch[k, h]) * P
        off = sch.cell_off[(k, h)]
        while n > 0:
            take = min(n, GCAP)
            # merge with previous call if same half and contiguous
            if calls and calls[-1][0] == h and \
                    calls[-1][1] + calls[-1][2] == off and \
                    calls[-1][2] + take <= GCAP:
                calls[-1] = (h, calls[-1][1], calls[-1][2] + take)
            else:
                calls.append((h, off, take))
            off += take
            n -= take
    # map chunk -> (call index, chunk offset within call)
    chunk_call = {}
    for idx_c, (h, off, n) in enumerate(calls):
        for j in range(n // P):
            chunk_call[off + j * P] = (idx_c, j)

    with tile.TileContext(nc) as tc:
        with tc.tile_pool(name="persist", bufs=1) as persist, \
             tc.tile_pool(name="dram", bufs=1, space="DRAM") as dram:
            idx_sb = persist.tile([P, sch.idx_total // 16], mybir.dt.int16)
            nc.sync.dma_start(idx_sb[:], idx_all[:])
            s_sb = persist.tile([P, sch.s_total], DT)
            nc.scalar.dma_start(s_sb[:], s_all[:])
            wh_sb = persist.tile([P, cfg.NLAYERS * P], DT)
            nc.sync.dma_start(wh_sb[:], w_hid[:])
            wo_sb = persist.tile([P, cfg.NCLASS], DT)
            nc.sync.dma_start(wo_sb[:], w_out[:])
            wp_sb = persist.tile([P, nk * P], DT)
            nc.sync.dma_start(wp_sb[:], w_proj[:])
            ones1 = persist.tile([1, P], f32)
            nc.vector.memset(ones1[:], 1.0)
            b_relu = persist.tile([P, 1], f32)
            nc.vector.memset(b_relu[:], B_RELU)
            idn = persist.tile([P, P], DT)
            make_identity(nc, idn[:])
            mm_sb = persist.tile([P, 2], f32)
            mm_ar = persist.tile([P, 2], f32)
            mm_back = persist.tile([1, 2], f32)
            sfac = persist.tile([P, 1], f32)
            bfac = persist.tile([P, 1], f32)
            x0sm = persist.tile([P, SMW], DT)   # slot-major x0 (beta-scaled)
            a1 = persist.tile([P, SMW], DT)     # slot-major activation buf
            # tail-tile pad lanes (partitions >= TAIL of the last 128-col
            # block) are never written by the tiled producers but are read
            # by full-width elementwise ops — zero them once.
            nc.vector.memset(x0sm[:, FT * P:], 0.0)
            nc.vector.memset(a1[:, FT * P:], 0.0)
            xact = persist.tile([P, SMW], DT)   # sigmoid scratch
            xn16 = persist.tile([P, SMW], DT)   # slot-major layer output

            def dma_half_a(eng, bnc, src):
                # src slot-major cols [0, SPLITA) -> bnc row-major
                eng.dma_start(
                    bnc[:].rearrange("(t p) h -> p t h", p=P),
                    src[:, :SPLITA].rearrange("p (t h) -> p t h", h=P))

            def dma_half_b(eng, bnc, src):
                # src slot-major cols [SPLITA, SMW) -> bnc row-major
                eng.dma_start(
                    bnc[:FT * P - SPLITA, :].rearrange("(t p) h -> p t h",
                                                       p=P),
                    src[:, SPLITA:FT * P].rearrange("p (t h) -> p t h", h=P))
                eng.dma_start(bnc[FT * P - SPLITA:SPLITB, :],
                              src[:TAIL, FT * P:FT * P + P])

            def exchange(li, src):
                dma_half_a(nc.sync, bounceA[li], src)
                nc.gpsimd.collective_compute(
                    "AllGather", ALU.bypass, ins=[bounceA[li].opt()],
                    outs=[x_fullA[li].opt()], replica_groups=rg)
                dma_half_b(nc.scalar, bounceB[li], src)
                nc.gpsimd.collective_compute(
                    "AllGather", ALU.bypass, ins=[bounceB[li].opt()],
                    outs=[x_fullB[li].opt()], replica_groups=rg)

            bounceA = [dram.tile([SPLITA, P], DT, name=f"bounceA{i}")
                       for i in range(NIT)]
            bounceB = [dram.tile([SPLITB, P], DT, name=f"bounceB{i}")
                       for i in range(NIT)]
            x_fullA = [dram.tile([8 * SPLITA, P], DT, addr_space="Shared",
                                 name=f"x_fullA{i}") for i in range(NIT)]
            x_fullB = [dram.tile([8 * SPLITB, P], DT, addr_space="Shared",
                                 name=f"x_fullB{i}") for i in range(NIT)]
            mm_in = dram.tile([1, 2], f32)
            mm_out = dram.tile([1, 2], f32, addr_space="Shared")

            # ================= projection =================
            with tc.tile_pool(name="strips", bufs=1) as strip_pool, \
                 tc.tile_pool(name="pwork", bufs=2) as pwork, \
                 tc.tile_pool(name="pps", bufs=2, space="PSUM") as pps_pool, \
                 tc.tile_pool(name="tps", bufs=4, space="PSUM") as tps_pool:
                strips = []
                for k in range(nk):
                    st = strip_pool.tile([P, R], DT, name=f"strip{k}",
                                         tag=f"strip{k}")
                    eng = nc.sync if k % 2 == 0 else nc.scalar
                    eng.dma_start(st[:], featT[k * P:(k + 1) * P, :])
                    strips.append(st)
                yT = strip_pool.tile([P, R], DT, name="yT", tag="yT")
                rmax = pwork.tile([P, 1], f32, name="rmax", tag="rmax")
                rmin = pwork.tile([P, 1], f32, name="rmin", tag="rmin")
                for ch in range(NCHK):
                    w = cw(ch)
                    sl = slice(ch * cfg.PACK, ch * cfg.PACK + w)
                    ps = pps_pool.tile([P, cfg.PACK], f32, name=f"h{ch}",
                                       tag="hps")
                    for k in range(nk):
                        nc.tensor.matmul(ps[:, :w],
                                         lhsT=wp_sb[:, k * P:(k + 1) * P],
                                         rhs=strips[k][:, sl],
                                         start=(k == 0), stop=(k == nk - 1))
                    nc.vector.tensor_copy(yT[:, sl], ps[:, :w])
                    for c2 in range(4 if w == cfg.PACK else (w + P - 1) // P):
                        w2 = min(P, w - c2 * P)
                        t2 = ch * 4 + c2
                        tp = tps_pool.tile([P, P], f32, name=f"x0t{t2}",
                                           tag="tps")
                        nc.tensor.matmul(
                            tp[:w2, :],
                            lhsT=yT[:, ch * cfg.PACK + c2 * P:
                                    ch * cfg.PACK + c2 * P + w2],
                            rhs=idn[:], start=True, stop=True)
                        nc.scalar.activation(x0sm[:w2, t2 * P:(t2 + 1) * P],
                                             tp[:w2, :], AFT.Copy)
                    qmax = pwork.tile([P, 1], f32, name="qmax", tag="qmax")
                    qmin = pwork.tile([P, 1], f32, name="qmin", tag="qmin")
                    nc.vector.tensor_reduce(qmax[:], ps[:, :w], axis=AX.X,
                                            op=ALU.max)
                    nc.vector.tensor_reduce(qmin[:], ps[:, :w], axis=AX.X,
                                            op=ALU.min)
                    if ch == 0:
                        nc.vector.tensor_copy(rmax[:], qmax[:])
                        nc.vector.tensor_copy(rmin[:], qmin[:])
                    else:
                        nc.vector.tensor_tensor(rmax[:], rmax[:], qmax[:],
                                                op=ALU.max)
                        nc.vector.tensor_tensor(rmin[:], rmin[:], qmin[:],
                                                op=ALU.min)
                nc.vector.tensor_copy(mm_sb[:, 0:1], rmax[:])
                nc.vector.tensor_scalar(mm_sb[:, 1:2], rmin[:], -1.0, None,
                                        ALU.mult)
                nc.gpsimd.partition_all_reduce(mm_ar[:], mm_sb[:],
                                               channels=P, reduce_op=RED.max)
                nc.sync.dma_start(mm_in[:], mm_ar[0:1, :])
                nc.gpsimd.collective_compute(
                    "AllReduce", ALU.max, ins=[mm_in.opt()],
                    outs=[mm_out.opt()], replica_groups=rg)
                nc.sync.dma_start(mm_back[:], mm_out[:])
                bc_ps = pps_pool.tile([P, 2], f32, name="bc_ps", tag="hps")
                nc.tensor.matmul(bc_ps[:], lhsT=ones1[:], rhs=mm_back[:],
                                 start=True, stop=True)
                bcast = pwork.tile([P, 2], f32, name="bcast", tag="qmin")
                nc.vector.tensor_copy(bcast[:], bc_ps[:])
                sden = pwork.tile([P, 1], f32, name="sden", tag="qmax")
                nc.vector.tensor_tensor(sden[:], bcast[:, 0:1], bcast[:, 1:2],
                                        op=ALU.add)
                nc.vector.tensor_scalar(sden[:], sden[:], 1e-8, None, ALU.add)
                nc.vector.reciprocal(sfac[:], sden[:])
                nc.vector.tensor_tensor(bfac[:], bcast[:, 1:2], sfac[:],
                                        op=ALU.mult)
                nc.vector.tensor_scalar(x0sm[:], x0sm[:], sfac[:], bfac[:],
                                        ALU.mult, ALU.add)
                exchange(0, x0sm)

            # ================= conv layers =================
            with tc.tile_pool(name="gpool", bufs=16) as gpool, \
                 tc.tile_pool(name="ywork", bufs=3) as ywork, \
                 tc.tile_pool(name="ow", bufs=2) as ow, \
                 tc.tile_pool(name="yps", bufs=3, space="PSUM") as yps_pool, \
                 tc.tile_pool(name="tps2", bufs=4, space="PSUM") as tps2_pool:
                beta_prev = 1.0
                for l in range(NIT):
                    last = l == NIT - 1
                    beta = min(0.5, (l + 1) / cfg.NLAYERS * 0.5)
                    c1 = float((1.0 - beta) * E1)
                    nc.vector.tensor_scalar(x0sm[:], x0sm[:],
                                            float(beta / beta_prev), None,
                                            ALU.mult)
                    beta_prev = beta
                    halves = [x_fullA[l][:], x_fullB[l][:]]
                    gts = []
                    if l == 0:
                        build_program._gq = 0
                    for (h, off, n) in calls:
                        gt = gpool.tile([P, GCAP], DT,
                                        name=f"g{l}_{off}", tag="g")
                        nc.gpsimd.dma_gather(
                            gt[:, :n].rearrange("p (n e) -> p n e", e=P),
                            halves[h],
                            idx_sb[:, off // 16:(off + n) // 16],
                            num_idxs=n, num_idxs_reg=n,
                            elem_size=P, queue_num=build_program._gq % 4)
                        build_program._gq += 1
                        gts.append(gt)
                    for k in range(NPACK):
                        wk = cw(k)
                        n_mm = sum(len(sch.s_cell[(k, h)]) for h in range(2))
                        ps = yps_pool.tile([P, cfg.PACK], f32,
                                           name=f"y{l}_{k}", tag="yps")
                        nc.vector.memset(ps[:, :wk], 0.0)
                        mi = 0
                        for h in range(2):
                            base = sch.cell_off[(k, h)]
                            for ci, (co, wdt, soff) in enumerate(
                                    sch.s_cell[(k, h)]):
                                cidx, j = chunk_call[base + ci * P]
                                lv = gts[cidx][:, j * P:(j + 1) * P]
                                wdt2 = min(wdt, wk - co)
                                nc.tensor.matmul(
                                    ps[:, co:co + wdt2],
                                    lhsT=lv,
                                    rhs=s_sb[:, soff:soff + wdt2],
                                    start=False, stop=(mi == n_mm - 1),
                                    skip_group_check=True)
                                mi += 1
                        yraw = ywork.tile([P, cfg.PACK], DT,
                                          name=f"yr{l}_{k}", tag="yr")
                        nc.vector.tensor_copy(yraw[:, :wk], ps[:, :wk])
                        # transform + transpose: per 128-dest chunk
                        nt_k = (wk + P - 1) // P
                        for c in range(nt_k):
                            w = min(P, wk - c * P)
                            gcol = k * cfg.PACK + c * P
                            tp = tps2_pool.tile([P, P], f32,
                                                name=f"t{l}_{k}_{c}",
                                                tag="tps2")
                            nc.tensor.matmul(
                                tp[:w, :], lhsT=yraw[:, c * P:c * P + w],
                                rhs=wh_sb[:, l * P:(l + 1) * P],
                                start=True, stop=True)
                            nc.scalar.activation(
                                a1[:w, gcol:gcol + P], tp[:w, :], AFT.Relu,
                                bias=b_relu[:w], scale=INV08)
                    # activation chain in two column halves so the A
                    # exchange launches while packs 6-12 still compute
                    for (c0, c1w) in ((0, SPLITA), (SPLITA, SMW - SPLITA)):
                        hs = slice(c0, c0 + c1w)
                        nc.vector.tensor_scalar(a1[:, hs], a1[:, hs], 1.0,
                                                c1, ALU.min, ALU.mult)
                        nc.scalar.activation(xact[:, hs], a1[:, hs],
                                             AFT.Sigmoid,
                                             scale=float(-1.0 / c1))
                        nc.vector.tensor_tensor(a1[:, hs], a1[:, hs],
                                                xact[:, hs], op=ALU.mult)
                        nc.vector.tensor_tensor(xn16[:, hs], a1[:, hs],
                                                x0sm[:, hs], op=ALU.add)
                        if not last:
                            if c0 == 0:
                                dma_half_a(nc.sync, bounceA[l + 1], xn16)
                                nc.gpsimd.collective_compute(
                                    "AllGather", ALU.bypass,
                                    ins=[bounceA[l + 1].opt()],
                                    outs=[x_fullA[l + 1].opt()],
                                    replica_groups=rg)
                            else:
                                dma_half_b(nc.scalar, bounceB[l + 1], xn16)
                                nc.gpsimd.collective_compute(
                                    "AllGather", ALU.bypass,
                                    ins=[bounceB[l + 1].opt()],
                                    outs=[x_fullB[l + 1].opt()],
                                    replica_groups=rg)

            # ================= output stage =================
            # EXPs are batched before a single LN so the scalar engine loads
            # each activation table once (table switches cost ~1.3us each).
            with tc.tile_pool(name="ow", bufs=4) as ow, \
                 tc.tile_pool(name="ops", bufs=4, space="PSUM") as ops_pool:
                lgall = ow.tile([P, NT * cfg.NCLASS], f32, name="lgall",
                                tag="lgall")
                sume49 = ow.tile([P, NT], f32, name="sume49", tag="sume49")
                lse49 = ow.tile([P, NT], f32, name="lse49", tag="lse49")
                nc.vector.memset(sume49[:], 1.0)
                for t in range(NT):
                    w = min(P, R - t * P)
                    tp = ops_pool.tile([P, P], f32, name=f"xt{t}", tag="oxt")
                    nc.tensor.matmul(
                        tp[:, :w],
                        lhsT=xn16[:w, t * P:(t + 1) * P],
                        rhs=idn[:w, :w], start=True, stop=True)
                    xnT = ow.tile([P, P], DT, name="xnT", tag="xnT")
                    nc.vector.tensor_copy(xnT[:, :w], tp[:, :w])
                    lg = ops_pool.tile([P, P], f32, name=f"lg{t}", tag="oxt")
                    nc.tensor.matmul(lg[:w, :cfg.NCLASS], lhsT=xnT[:, :w],
                                     rhs=wo_sb[:], start=True, stop=True)
                    nc.vector.tensor_copy(
                        lgall[:w, t * cfg.NCLASS:(t + 1) * cfg.NCLASS],
                        lg[:w, :cfg.NCLASS])
                    # logits are bounded (xn in [0,1], small W_out), so
                    # exp/sum is stable without the max subtraction
                    ex = ow.tile([P, cfg.NCLASS], f32, name="ex", tag="ex")
                    nc.scalar.activation(
                        ex[:w], lgall[:w, t * cfg.NCLASS:(t + 1) * cfg.NCLASS],
                        AFT.Exp, accum_out=sume49[:w, t:t + 1])
                nc.scalar.activation(lse49[:], sume49[:], AFT.Ln)
                for t in range(NT):
                    w = min(P, R - t * P)
                    res = ow.tile([P, cfg.NCLASS], f32, name="res", tag="ex")
                    nc.vector.tensor_scalar(
                        res[:w], lgall[:w, t * cfg.NCLASS:(t + 1) * cfg.NCLASS],
                        lse49[:w, t:t + 1], None, ALU.subtract)
                    nc.sync.dma_start(out[t * P:t * P + w, :], res[:w])
    nc.compile()
    return nc


def kernel(**inputs) -> np.ndarray:
    cfg = Cfg()
    features = np.asarray(inputs["features"], np.float32)
    edge_row = np.asarray(inputs["edge_row"], np.int64)
    edge_col = np.asarray(inputs["edge_col"], np.int64)
    W_in = np.asarray(inputs["W_in"], np.float32)
    Ws = np.asarray(inputs["Ws"], np.float32)
    c = np.asarray(inputs["c"], np.float32)
    W_out = np.asarray(inputs["W_out"], np.float32)

    in_maps, sch, dest_of = preprocess(cfg, features, edge_row, edge_col,
                                       W_in, Ws, c, W_out)
    nc = build_program(cfg, sch)

    import os
    from concourse import bass_utils
    res = bass_utils.run_bass_kernel_spmd(
        nc, in_maps, core_ids=list(range(cfg.n_cores)),
        trace=bool(os.environ.get("GNN_TRACE")))
    kernel.last_result = res
    out = np.empty((cfg.N, cfg.NCLASS), np.float32)
    for d in range(cfg.n_cores):
        out[d * cfg.R + dest_of[d]] = res.results[d]["out"]
    return out


# revision 27
# speedup vs baseline: 1.0229x; 1.0229x over previous
"""Trainium2 Bass kernel for nn_NewActivationGNN (GNN message passing).

Architecture (v2, swdge gather):
  y_l = A_norm @ (x @ W_l) is computed as (A_norm @ x) @ W_l — aggregate
  raw hidden rows first, transform after. Per-edge source rows are pulled
  straight from the DRAM-resident AllGathered node table with SWDGE
  dma_gather (one 256B descriptor per edge, slot-major destination), so
  the gather costs ~23ns/edge across 16 SDMA engines instead of ~26ns/edge
  serialized on gpsimd Q7 cores.

  Slot-major pipeline: gathered chunks [128 edge-slots, 128 hid] are lhsT
  for selection-matrix matmuls (rhs = S fp16 [slot, dest-span] carrying
  deg_inv) accumulating feature-major y_raw per 512-dest pack in PSUM;
  the W_l transform (lhsT = y_raw chunk, rhs = W) lands slot-major
  [128 dest, 128 hid] tiles; activation + residual run slot-major
  full-width; the next table is DMA'd out row-major and AllGathered.

  S matrices and the index stream are small (dest-dense cells of
  (pack, src-half)) and stay SBUF-resident across all 4 layers.

Sharding: nodes split across 8 cores by destination (graph parallel);
per-layer AllGather of the fp16 slot-major table; weights replicated.
SPMD: one program for all cores; per-device variation (indices, S
matrices, features) is data. Chunk counts / S spans are made
device-uniform by padding to the cross-device maximum.

Edge indices are int16, so the node table is addressed in two halves
(src position < 32768 vs >= 32768); cells are keyed (pack, half).
"""

import sys

for _p in ("/opt/trn_rl_repo", "/root/.axon_site/_ro/trn_rl_repo"):
    if _p not in sys.path:
        sys.path.insert(0, _p)

from dataclasses import dataclass

import numpy as np

import concourse.bass as bass  # noqa: F401
import concourse.tile as tile
from concourse import bacc, mybir
from concourse.masks import make_identity

P = 128
HALF = 32768  # int16 index range per table half


@dataclass
class Cfg:
    N: int = 50000
    E: int = 800000
    NFEAT: int = 500
    NHID: int = 128
    NCLASS: int = 40
    NLAYERS: int = 4
    GAMMA: float = 0.3
    X1: float = 0.1
    X2: float = 0.9
    C_ACT: float = -1.0
    n_cores: int = 8
    PACK: int = 512

    @property
    def R(self):
        return self.N // self.n_cores          # 6250 dest rows per core

    @property
    def NPACK(self):
        return (self.R + self.PACK - 1) // self.PACK   # 13

    @property
    def NFP(self):
        return ((self.NFEAT + 1 + P - 1) // P) * P     # 512


class Sched:
    """Device-uniform schedule over cells keyed (pack k, half h)."""

    def __init__(self, cfg, counts, spans):
        self.B = counts.max(axis=0)                      # [NPACK, 2]
        self.nch = (self.B + P - 1) // P
        self.cell_off = {}
        off = 0
        for k in range(cfg.NPACK):
            for h in range(2):
                self.cell_off[(k, h)] = off
                off += int(self.nch[k, h]) * P
        self.idx_total = off
        s_off = 0
        self.s_cell = {}
        for k in range(cfg.NPACK):
            for h in range(2):
                metas = []
                for ci in range(int(self.nch[k, h])):
                    co, w = spans[(k, h, ci)]
                    metas.append((co, w, s_off))
                    s_off += w
                self.s_cell[(k, h)] = metas
        self.s_total = s_off


def preprocess(cfg: Cfg, features, edge_row, edge_col, W_in, Ws, c, W_out):
    N, R, ncores = cfg.N, cfg.R, cfg.n_cores
    NPACK, PACK = cfg.NPACK, cfg.PACK
    f32 = np.float32

    deg = np.bincount(edge_row, minlength=N)
    deg_inv = (1.0 / np.maximum(deg, 1)).astype(f32)
    owner = edge_row // R

    # Node permutation: within each device, order nodes by descending global
    # degree so cumulative degree profiles align across devices (smaller S
    # spans, less cell padding). pos[] maps orig node id -> its table
    # position within the owning device's slice.
    dest_of = np.empty((ncores, R), np.int64)   # sorted pos -> orig local
    pos = np.empty(N, np.int64)
    for d in range(ncores):
        order = np.argsort(-deg[d * R:(d + 1) * R], kind="stable")
        dest_of[d] = order
        local_of = np.empty(R, np.int64)
        local_of[order] = np.arange(R)
        pos[d * R:(d + 1) * R] = local_of

    dev = []
    counts = np.zeros((ncores, NPACK, 2), np.int64)
    for d in range(ncores):
        m = owner == d
        dl = pos[edge_row[m]]
        src = edge_col[m].astype(np.int64)
        gp = (src // R) * R + pos[src]          # global table position
        h = gp // HALF                          # table half (0 or 1)
        iv = gp - h * HALF                      # int16-safe row index
        k = dl // PACK
        so = np.lexsort((dl, h, k))
        dl, h, iv, k = dl[so], h[so], iv[so], k[so]
        cell_id = k * 2 + h
        cnt = np.bincount(cell_id, minlength=NPACK * 2)
        counts[d] = cnt.reshape(NPACK, 2)
        dev.append((dl, iv, cell_id))

    B = counts.max(axis=0)
    nch = (B + P - 1) // P

    spans = {}
    dev_cell_start = []
    for d in range(ncores):
        cnt = counts[d].reshape(-1)
        dev_cell_start.append(np.concatenate([[0], np.cumsum(cnt)]))
    for k in range(NPACK):
        for h in range(2):
            cid = k * 2 + h
            for ci in range(int(nch[k, h])):
                fd, ld = [], []
                for d in range(ncores):
                    dl = dev[d][0]
                    b = dev_cell_start[d][cid]
                    n_d = counts[d, k, h]
                    s0 = ci * P
                    if s0 < n_d:
                        s1 = min(s0 + P, n_d)
                        fd.append(int(dl[b + s0]))
                        ld.append(int(dl[b + s1 - 1]))
                co = min(fd) - k * PACK
                w = max(ld) - min(fd) + 1
                spans[(k, h, ci)] = (co, w)

    sch = Sched(cfg, counts, spans)

    # shared weights
    NFP = cfg.NFP
    W_aug = np.zeros((NFP, cfg.NHID), f32)
    W_aug[:cfg.NFEAT] = (1.0 - cfg.GAMMA) * W_in
    W_aug[cfg.NFEAT] = cfg.GAMMA * np.maximum(c, 0.0)
    nk = NFP // P
    W_dram = np.empty((P, nk * P), np.float16)
    for kk in range(nk):
        W_dram[:, kk * P:(kk + 1) * P] = W_aug[kk * P:(kk + 1) * P]
    Ws_dram = np.empty((P, cfg.NLAYERS * P), np.float16)
    for l in range(cfg.NLAYERS):
        Ws_dram[:, l * P:(l + 1) * P] = Ws[l]
    Wout_dram = np.ascontiguousarray(W_out).astype(np.float16)

    in_maps = []
    for d in range(ncores):
        dl, iv, cell_id = dev[d]
        cstart = dev_cell_start[d]
        idx_vals = np.zeros(sch.idx_total, np.int16)
        s_data = np.zeros((P, sch.s_total), np.float16)
        for k in range(NPACK):
            for h in range(2):
                cid = k * 2 + h
                n_d = int(counts[d, k, h])
                if n_d == 0:
                    continue
                b = cstart[cid]
                o = sch.cell_off[(k, h)]
                idx_vals[o:o + n_d] = iv[b:b + n_d].astype(np.int16)
                metas = sch.s_cell[(k, h)]
                sl = np.arange(n_d)
                ci_arr = sl // P
                row = sl % P
                co = np.array([m[0] for m in metas], np.int64)[ci_arr]
                soff = np.array([m[2] for m in metas], np.int64)[ci_arr]
                col = soff + (dl[b:b + n_d] - k * PACK - co)
                s_data[row, col] = deg_inv[
                    d * R + dest_of[d][dl[b:b + n_d]]]
        idx_t = np.tile(idx_vals.reshape(-1, 16).T, (8, 1))

        gids = d * R + dest_of[d]
        featT = np.zeros((NFP, R), np.float16)
        featT[:cfg.NFEAT] = features[gids].T
        featT[cfg.NFEAT] = 1.0

        in_maps.append(dict(
            featT=featT, idx_all=np.ascontiguousarray(idx_t), s_all=s_data,
            w_proj=W_dram, w_hid=Ws_dram, w_out=Wout_dram,
        ))
    return in_maps, sch, dest_of


def build_program(cfg: Cfg, sch: Sched):
    nc = bacc.Bacc("TRN2", target_bir_lowering=False, debug=False,
                   num_devices=cfg.n_cores, num_swdge_queues=4)
    DT = mybir.dt.float16
    f32 = mybir.dt.float32
    R, NPACK, NFP = cfg.R, cfg.NPACK, cfg.NFP
    AFT = mybir.ActivationFunctionType
    ALU = mybir.AluOpType
    AX = mybir.AxisListType
    RED = bass.bass_isa.ReduceOp
    rg = [list(range(cfg.n_cores))]
    nk = NFP // P
    NCHK = (R + cfg.PACK - 1) // cfg.PACK   # 512-col chunks over R
    NT = R // P + (1 if R % P else 0)       # 128-col tiles over R (49)

    def cw(ch):
        return min(cfg.PACK, R - ch * cfg.PACK)

    featT = nc.dram_tensor("featT", [NFP, R], DT, kind="ExternalInput").ap()
    idx_all = nc.dram_tensor("idx_all", [P, sch.idx_total // 16],
                             mybir.dt.int16, kind="ExternalInput").ap()
    s_all = nc.dram_tensor("s_all", [P, sch.s_total], DT,
                           kind="ExternalInput").ap()
    w_proj = nc.dram_tensor("w_proj", [P, nk * P], DT,
                            kind="ExternalInput").ap()
    w_hid = nc.dram_tensor("w_hid", [P, cfg.NLAYERS * P], DT,
                           kind="ExternalInput").ap()
    w_out = nc.dram_tensor("w_out", [P, cfg.NCLASS], DT,
                           kind="ExternalInput").ap()
    out = nc.dram_tensor("out", [R, cfg.NCLASS], f32,
                         kind="ExternalOutput").ap()

    INV08 = float(np.float32(1.0 / (np.float64(cfg.X2) - cfg.X1 + 1e-8)))
    B_RELU = float(np.float32(-cfg.X1 * INV08))
    E1 = float(1.0 + np.exp(-cfg.C_ACT))
    NIT = cfg.NLAYERS
    FT = R // P          # full 128-row tiles (48)
    TAIL = R - FT * P    # ragged tail rows (106)
    SMW = NT * P         # slot-major width (6272)

    # gather calls: group cells into calls of <= GCAP slots (contiguous in
    # the idx stream). Each call is one dma_gather against one table half.
    # GCAP is bounded by the SWDGE descriptor-ring carveout: one call's
    # descriptors (num_idxs/16 + 1 per DMA engine) must fit the ring or the
    # Q7 waits forever (hardware-verified: 1024 ok, 2560 wedges).
    GCAP = 1024
    calls = []     # (half, slot_off, n_slots)
    for k in range(NPACK):
        for h in range(2):
            n = int(sch.nch[k, h]) * P
            off = sch.cell_off[(k, h)]
            while n > 0:
                take = min(n, GCAP)
                # merge with previous call if same half and contiguous
                if calls and calls[-1][0] == h and \
                        calls[-1][1] + calls[-1][2] == off and \
                        calls[-1][2] + take <= GCAP:
                    calls[-1] = (h, calls[-1][1], calls[-1][2] + take)
                else:
                    calls.append((h, off, take))
                off += take
                n -= take
    # map chunk -> (call index, chunk offset within call)
    chunk_call = {}
    for idx_c, (h, off, n) in enumerate(calls):
        for j in range(n // P):
            chunk_call[off + j * P] = (idx_c, j)

    with tile.TileContext(nc) as tc:
        with tc.tile_pool(name="persist", bufs=1) as persist, \
             tc.tile_pool(name="dram", bufs=1, space="DRAM") as dram:
            idx_sb = persist.tile([P, sch.idx_total // 16], mybir.dt.int16)
            nc.sync.dma_start(idx_sb[:], idx_all[:])
            s_sb = persist.tile([P, sch.s_total], DT)
            nc.scalar.dma_start(s_sb[:], s_all[:])
            wh_sb = persist.tile([P, cfg.NLAYERS * P], DT)
            nc.sync.dma_start(wh_sb[:], w_hid[:])
            wo_sb = persist.tile([P, cfg.NCLASS], DT)
            nc.sync.dma_start(wo_sb[:], w_out[:])
            wp_sb = persist.tile([P, nk * P], DT)
            nc.sync.dma_start(wp_sb[:], w_proj[:])
            ones1 = persist.tile([1, P], f32)
            nc.vector.memset(ones1[:], 1.0)
            b_relu = persist.tile([P, 1], f32)
            nc.vector.memset(b_relu[:], B_RELU)
            idn = persist.tile([P, P], DT)
            make_identity(nc, idn[:])
            mm_sb = persist.tile([P, 2], f32)
            mm_ar = persist.tile([P, 2], f32)
            mm_back = persist.tile([1, 2], f32)
            sfac = persist.tile([P, 1], f32)
            bfac = persist.tile([P, 1], f32)
            x0sm = persist.tile([P, SMW], DT)   # slot-major x0 (beta-scaled)
            a1 = persist.tile([P, SMW], DT)     # slot-major activation buf
            # tail-tile pad lanes (partitions >= TAIL of the last 128-col
            # block) are never written by the tiled producers but are read
            # by full-width elementwise ops — zero them once.
            nc.vector.memset(x0sm[:, FT * P:], 0.0)
            nc.vector.memset(a1[:, FT * P:], 0.0)
            xact = persist.tile([P, SMW], DT)   # sigmoid scratch
            xn16 = persist.tile([P, SMW], DT)   # slot-major layer output

            def dma_slot_major_out(eng, bnc, src):
                # src slot-major [P, SMW] -> bnc row-major [R, P]
                eng.dma_start(
                    bnc[:FT * P, :].rearrange("(t p) h -> p t h", p=P),
                    src[:, :FT * P].rearrange("p (t h) -> p t h", h=P))
                eng.dma_start(bnc[FT * P:R, :], src[:TAIL, FT * P:FT * P + P])

            bounce = [dram.tile([R, P], DT, name=f"bounce{i}")
                      for i in range(NIT)]
            x_full = [dram.tile([cfg.N, P], DT, addr_space="Shared",
                                name=f"x_full{i}") for i in range(NIT)]
            mm_in = dram.tile([1, 2], f32)
            mm_out = dram.tile([1, 2], f32, addr_space="Shared")

            # ================= projection =================
            with tc.tile_pool(name="strips", bufs=1) as strip_pool, \
                 tc.tile_pool(name="pwork", bufs=2) as pwork, \
                 tc.tile_pool(name="pps", bufs=2, space="PSUM") as pps_pool, \
                 tc.tile_pool(name="tps", bufs=4, space="PSUM") as tps_pool:
                strips = []
                for k in range(nk):
                    st = strip_pool.tile([P, R], DT, name=f"strip{k}",
                                         tag=f"strip{k}")
                    eng = nc.sync if k % 2 == 0 else nc.scalar
                    eng.dma_start(st[:], featT[k * P:(k + 1) * P, :])
                    strips.append(st)
                yT = strip_pool.tile([P, R], DT, name="yT", tag="yT")
                rmax = pwork.tile([P, 1], f32, name="rmax", tag="rmax")
                rmin = pwork.tile([P, 1], f32, name="rmin", tag="rmin")
                for ch in range(NCHK):
                    w = cw(ch)
                    sl = slice(ch * cfg.PACK, ch * cfg.PACK + w)
                    ps = pps_pool.tile([P, cfg.PACK], f32, name=f"h{ch}",
                                       tag="hps")
                    for k in range(nk):
                        nc.tensor.matmul(ps[:, :w],
                                         lhsT=wp_sb[:, k * P:(k + 1) * P],
                                         rhs=strips[k][:, sl],
                                         start=(k == 0), stop=(k == nk - 1))
                    nc.vector.tensor_copy(yT[:, sl], ps[:, :w])
                    for c2 in range(4 if w == cfg.PACK else (w + P - 1) // P):
                        w2 = min(P, w - c2 * P)
                        t2 = ch * 4 + c2
                        tp = tps_pool.tile([P, P], f32, name=f"x0t{t2}",
                                           tag="tps")
                        nc.tensor.matmul(
                            tp[:w2, :],
                            lhsT=yT[:, ch * cfg.PACK + c2 * P:
                                    ch * cfg.PACK + c2 * P + w2],
                            rhs=idn[:], start=True, stop=True)
                        nc.scalar.activation(x0sm[:w2, t2 * P:(t2 + 1) * P],
                                             tp[:w2, :], AFT.Copy)
                    qmax = pwork.tile([P, 1], f32, name="qmax", tag="qmax")
                    qmin = pwork.tile([P, 1], f32, name="qmin", tag="qmin")
                    nc.vector.tensor_reduce(qmax[:], ps[:, :w], axis=AX.X,
                                            op=ALU.max)
                    nc.vector.tensor_reduce(qmin[:], ps[:, :w], axis=AX.X,
                                            op=ALU.min)
                    if ch == 0:
                        nc.vector.tensor_copy(rmax[:], qmax[:])
                        nc.vector.tensor_copy(rmin[:], qmin[:])
                    else:
                        nc.vector.tensor_tensor(rmax[:], rmax[:], qmax[:],
                                                op=ALU.max)
                        nc.vector.tensor_tensor(rmin[:], rmin[:], qmin[:],
                                                op=ALU.min)
                nc.vector.tensor_copy(mm_sb[:, 0:1], rmax[:])
                nc.vector.tensor_scalar(mm_sb[:, 1:2], rmin[:], -1.0, None,
                                        ALU.mult)
                nc.gpsimd.partition_all_reduce(mm_ar[:], mm_sb[:],
                                               channels=P, reduce_op=RED.max)
                nc.sync.dma_start(mm_in[:], mm_ar[0:1, :])
                nc.gpsimd.collective_compute(
                    "AllReduce", ALU.max, ins=[mm_in.opt()],
                    outs=[mm_out.opt()], replica_groups=rg)
                nc.sync.dma_start(mm_back[:], mm_out[:])
                bc_ps = pps_pool.tile([P, 2], f32, name="bc_ps", tag="hps")
                nc.tensor.matmul(bc_ps[:], lhsT=ones1[:], rhs=mm_back[:],
                                 start=True, stop=True)
                bcast = pwork.tile([P, 2], f32, name="bcast", tag="qmin")
                nc.vector.tensor_copy(bcast[:], bc_ps[:])
                sden = pwork.tile([P, 1], f32, name="sden", tag="qmax")
                nc.vector.tensor_tensor(sden[:], bcast[:, 0:1], bcast[:, 1:2],
                                        op=ALU.add)
                nc.vector.tensor_scalar(sden[:], sden[:], 1e-8, None, ALU.add)
                nc.vector.reciprocal(sfac[:], sden[:])
                nc.vector.tensor_tensor(bfac[:], bcast[:, 1:2], sfac[:],
                                        op=ALU.mult)
                nc.vector.tensor_scalar(x0sm[:], x0sm[:], sfac[:], bfac[:],
                                        ALU.mult, ALU.add)
                dma_slot_major_out(nc.sync, bounce[0], x0sm)
            nc.gpsimd.collective_compute(
                "AllGather", ALU.bypass, ins=[bounce[0].opt()],
                outs=[x_full[0].opt()], replica_groups=rg)

            # ================= conv layers =================
            with tc.tile_pool(name="gpool", bufs=16) as gpool, \
                 tc.tile_pool(name="ywork", bufs=3) as ywork, \
                 tc.tile_pool(name="ow", bufs=2) as ow, \
                 tc.tile_pool(name="yps", bufs=3, space="PSUM") as yps_pool, \
                 tc.tile_pool(name="tps2", bufs=4, space="PSUM") as tps2_pool:
                beta_prev = 1.0
                for l in range(NIT):
                    last = l == NIT - 1
                    beta = min(0.5, (l + 1) / cfg.NLAYERS * 0.5)
                    c1 = float((1.0 - beta) * E1)
                    nc.vector.tensor_scalar(x0sm[:], x0sm[:],
                                            float(beta / beta_prev), None,
                                            ALU.mult)
                    beta_prev = beta
                    xf = x_full[l][:]
                    halves = [xf[:HALF, :], xf[HALF:, :]]
                    gts = []
                    if l == 0:
                        build_program._gq = 0
                    for (h, off, n) in calls:
                        gt = gpool.tile([P, GCAP], DT,
                                        name=f"g{l}_{off}", tag="g")
                        nc.gpsimd.dma_gather(
                            gt[:, :n].rearrange("p (n e) -> p n e", e=P),
                            halves[h],
                            idx_sb[:, off // 16:(off + n) // 16],
                            num_idxs=n, num_idxs_reg=n,
                            elem_size=P, queue_num=build_program._gq % 4)
                        build_program._gq += 1
                        gts.append(gt)
                    for k in range(NPACK):
                        wk = cw(k)
                        n_mm = sum(len(sch.s_cell[(k, h)]) for h in range(2))
                        ps = yps_pool.tile([P, cfg.PACK], f32,
                                           name=f"y{l}_{k}", tag="yps")
                        nc.vector.memset(ps[:, :wk], 0.0)
                        mi = 0
                        for h in range(2):
                            base = sch.cell_off[(k, h)]
                            for ci, (co, wdt, soff) in enumerate(
                                    sch.s_cell[(k, h)]):
                                cidx, j = chunk_call[base + ci * P]
                                lv = gts[cidx][:, j * P:(j + 1) * P]
                                wdt2 = min(wdt, wk - co)
                                nc.tensor.matmul(
                                    ps[:, co:co + wdt2],
                                    lhsT=lv,
                                    rhs=s_sb[:, soff:soff + wdt2],
                                    start=False, stop=(mi == n_mm - 1),
                                    skip_group_check=True)
                                mi += 1
                        yraw = ywork.tile([P, cfg.PACK], DT,
                                          name=f"yr{l}_{k}", tag="yr")
                        nc.vector.tensor_copy(yraw[:, :wk], ps[:, :wk])
                        # transform + transpose: per 128-dest chunk
                        nt_k = (wk + P - 1) // P
                        for c in range(nt_k):
                            w = min(P, wk - c * P)
                            gcol = k * cfg.PACK + c * P
                            tp = tps2_pool.tile([P, P], f32,
                                                name=f"t{l}_{k}_{c}",
                                                tag="tps2")
                            nc.tensor.matmul(
                                tp[:w, :], lhsT=yraw[:, c * P:c * P + w],
                                rhs=wh_sb[:, l * P:(l + 1) * P],
                                start=True, stop=True)
                            nc.scalar.activation(
                                a1[:w, gcol:gcol + P], tp[:w, :], AFT.Relu,
                                bias=b_relu[:w], scale=INV08)
                    # activation chain (slot-major, full width, fp16)
                    nc.vector.tensor_scalar(a1[:], a1[:], 1.0, c1,
                                            ALU.min, ALU.mult)
                    nc.scalar.activation(xact[:], a1[:], AFT.Sigmoid,
                                         scale=float(-1.0 / c1))
                    nc.vector.tensor_tensor(a1[:], a1[:], xact[:],
                                            op=ALU.mult)
                    nc.vector.tensor_tensor(xn16[:], a1[:], x0sm[:],
                                            op=ALU.add)
                    if not last:
                        dma_slot_major_out(nc.sync, bounce[l + 1], xn16)
                        nc.gpsimd.collective_compute(
                            "AllGather", ALU.bypass, ins=[bounce[l + 1].opt()],
                            outs=[x_full[l + 1].opt()], replica_groups=rg)

            # ================= output stage =================
            # EXPs are batched before a single LN so the scalar engine loads
            # each activation table once (table switches cost ~1.3us each).
            with tc.tile_pool(name="ow", bufs=4) as ow, \
                 tc.tile_pool(name="ops", bufs=4, space="PSUM") as ops_pool:
                lgall = ow.tile([P, NT * cfg.NCLASS], f32, name="lgall",
                                tag="lgall")
                sume49 = ow.tile([P, NT], f32, name="sume49", tag="sume49")
                lse49 = ow.tile([P, NT], f32, name="lse49", tag="lse49")
                nc.vector.memset(sume49[:], 1.0)
                for t in range(NT):
                    w = min(P, R - t * P)
                    tp = ops_pool.tile([P, P], f32, name=f"xt{t}", tag="oxt")
                    nc.tensor.matmul(
                        tp[:, :w],
                        lhsT=xn16[:w, t * P:(t + 1) * P],
                        rhs=idn[:w, :w], start=True, stop=True)
                    xnT = ow.tile([P, P], DT, name="xnT", tag="xnT")
                    nc.vector.tensor_copy(xnT[:, :w], tp[:, :w])
                    lg = ops_pool.tile([P, P], f32, name=f"lg{t}", tag="oxt")
                    nc.tensor.matmul(lg[:w, :cfg.NCLASS], lhsT=xnT[:, :w],
                                     rhs=wo_sb[:], start=True, stop=True)
                    nc.vector.tensor_copy(
                        lgall[:w, t * cfg.NCLASS:(t + 1) * cfg.NCLASS],
                        lg[:w, :cfg.NCLASS])
                    # logits are bounded (xn in [0,1], small W_out), so
                    # exp/sum is stable without the max subtraction
                    ex = ow.tile([P, cfg.NCLASS], f32, name="ex", tag="ex")
                    nc.scalar.activation(
                        ex[:w], lgall[:w, t * cfg.NCLASS:(t + 1) * cfg.NCLASS],
                        AFT.Exp, accum_out=sume49[:w, t:t + 1])
                nc.scalar.activation(lse49[:], sume49[:], AFT.Ln)
                for t in range(NT):
                    w = min(P, R - t * P)
                    res = ow.tile([P, cfg.NCLASS], f32, name="res", tag="ex")
                    nc.vector.tensor_scalar(
                        res[:w], lgall[:w, t * cfg.NCLASS:(t + 1) * cfg.NCLASS],
                        lse49[:w, t:t + 1], None, ALU.subtract)
                    nc.sync.dma_start(out[t * P:t * P + w, :], res[:w])
    nc.compile()
    return nc


def kernel(**inputs) -> np.ndarray:
    cfg = Cfg()
    features = np.asarray(inputs["features"], np.float32)
    edge_row = np.asarray(inputs["edge_row"], np.int64)
    edge_col = np.asarray(inputs["edge_col"], np.int64)
    W_in = np.asarray(inputs["W_in"], np.float32)
    Ws = np.asarray(inputs["Ws"], np.float32)
    c = np.asarray(inputs["c"], np.float32)
    W_out = np.asarray(inputs["W_out"], np.float32)

    in_maps, sch, dest_of = preprocess(cfg, features, edge_row, edge_col,
                                       W_in, Ws, c, W_out)
    nc = build_program(cfg, sch)

    import os
    from concourse import bass_utils
    res = bass_utils.run_bass_kernel_spmd(
        nc, in_maps, core_ids=list(range(cfg.n_cores)),
        trace=bool(os.environ.get("GNN_TRACE")))
    kernel.last_result = res
    out = np.empty((cfg.N, cfg.NCLASS), np.float32)
    for d in range(cfg.n_cores):
        out[d * cfg.R + dest_of[d]] = res.results[d]["out"]
    return out
